# revision 18
# baseline (speedup 1.0000x reference)
"""Cumulative LayerNorm (B=4, C=512, T=32000) on 8 Trainium2 NeuronCores.

Sharding: core j handles batch b = j//2, T-half h = j%2.
Cumulative stats over (C, 0..t) need a carry from the first T-half; each
h=1 core recomputes it with a prefix reduce over that half (no cross-core
comm). SPMD symmetry: h=0 cores run the same prefix pass on their own
data and multiply the carry by flag=0.

Per-core pipeline, one streaming pass per 3200-t segment:
  - 4 big DMAs (1.6MB, 12.8KB/partition lines) load the segment
  - fp32->bf16 converts (split ScalarE/GpSimd), bf16 squares (ScalarE)
  - per-t channel sums: M=2 matmuls (lhsT e0/e1 put s on partition 0,
    q on partition 1 of one PSUM bank; LDWEIGHTS ~free at 2 columns)
  - running cumsum: tensor_tensor_scan on the PSUM rows, chained across
    all tiles by `initial` APs (VectorE)
  - per segment: one DMA reshapes [2,3200] rows to t-major [128,50];
    stats finalized there (mean/var/recip-sqrt, tiny [128,25] ops);
    f32r-rounded inv and -mean*inv reshaped back to rows
  - A=ones x inv, B=ones x (-mean*inv) K=1 f32r matmuls -> PSUM [128,400]
  - y = x*A + B in place on the x segment: 2 TT ops per 400-block with
    cb-repeat PSUM APs (VectorE)
  - 4 big DMAs store the segment
"""
import numpy as np

import concourse.bass as bass
import concourse.bacc as bacc
import concourse.tile as tile
from concourse import mybir
from concourse.bass_utils import run_bass_kernel_spmd

F32 = mybir.dt.float32
F32R = mybir.dt.float32r
BF16 = mybir.dt.bfloat16

B, C, T = 4, 512, 32000
NCORES = 8
TH = T // 2          # 16000 per core
CB = C // 128        # 4 channel blocks
SEG = 3200           # segment
F = SEG // 128       # 25 (t-major free dim per segment)
NSEG = TH // SEG     # 5
TS = 400             # stats matmul tile (N=400 moving cols)
NTS = SEG // TS      # 8
TN = 400             # normalize block (A/B psum [128, 400])
NTN = SEG // TN      # 8
QS = 800             # convert/square op granularity (quarter segment)
NQS = SEG // QS      # 4
EPS = 1e-08

_CACHE = {}


def _build(wb_general: bool):
    nc = bacc.Bacc()

    xc_e = nc.declare_dram_parameter("xc", [C, TH], F32, isOutput=False)
    xp_e = nc.declare_dram_parameter("xp", [C, TH], F32, isOutput=False)
    flag_e = nc.declare_dram_parameter("flag", [1, 1], F32, isOutput=False)
    invp_e = nc.declare_dram_parameter("invp", [128, F * NSEG], F32, isOutput=False)
    invm_e = nc.declare_dram_parameter("invm", [128, F * NSEG], F32, isOutput=False)
    w_e = nc.declare_dram_parameter("w", [1, C], F32, isOutput=False)
    b_e = nc.declare_dram_parameter("b", [1, C], F32, isOutput=False)
    y_e = nc.declare_dram_parameter("y", [C, TH], F32, isOutput=True)

    xc_r = xc_e.rearrange("(cb p) t -> cb p t", p=128)
    xp_r = xp_e.rearrange("(cb p) t -> cb p t", p=128)
    y_r = y_e.rearrange("(cb p) t -> cb p t", p=128)

    with tile.TileContext(nc) as tc:
        with (
            tc.tile_pool(name="misc", bufs=1) as misc,
            tc.tile_pool(name="xin", bufs=2) as xin,
            tc.tile_pool(name="bfp", bufs=2) as bfp,
            tc.tile_pool(name="rows", bufs=1) as rows,
            tc.tile_pool(name="segc", bufs=2) as segc,
            tc.tile_pool(name="abrow", bufs=1) as abrow,
            tc.tile_pool(name="tmaj", bufs=2) as tmaj,
            tc.tile_pool(name="dram", bufs=3, space="DRAM") as dram,
        ):
            # ---- constants
            # M=1 ones column: stats land on partition 0, LDWEIGHTS ~free
            ones1 = misc.tile([128, 1], BF16, tag="ones1")
            nc.vector.memset(ones1, 1.0)
            ones1f = misc.tile([128, 1], F32, tag="ones1f")
            nc.vector.memset(ones1f, 1.0)
            ones_f = misc.tile([1, 128], F32, tag="ones_f")
            nc.vector.memset(ones_f, 1.0)
            ones_r = misc.tile([1, 128], F32R, tag="ones_r")
            nc.scalar.copy(out=ones_r, in_=ones_f)
            eps_t = misc.tile([128, 1], F32, tag="eps_t")
            nc.vector.memset(eps_t, EPS)
            zrow = misc.tile([1, TS], F32, tag="zrow")
            nc.vector.memset(zrow, 0.0)
            flag_t = misc.tile([1, 1], F32, tag="flag_t")
            nc.sync.dma_start(out=flag_t, in_=flag_e[:, :])
            invp_t = misc.tile([128, F * NSEG], F32, tag="invp_t")
            nc.sync.dma_start(out=invp_t, in_=invp_e[:, :])
            invm_t = misc.tile([128, F * NSEG], F32, tag="invm_t")
            nc.sync.dma_start(out=invm_t, in_=invm_e[:, :])
            carry_s = misc.tile([1, 1], F32, tag="carry_s")
            carry_q = misc.tile([1, 1], F32, tag="carry_q")
            if wb_general:
                wcol = misc.tile([128, CB], F32, tag="wcol")
                bcol = misc.tile([128, CB], F32, tag="bcol")
                for cb in range(CB):
                    nc.sync.dma_start(
                        out=wcol[:, cb : cb + 1],
                        in_=w_e[0:1, cb * 128 : (cb + 1) * 128].rearrange(
                            "one p -> (one p) 1"
                        ),
                    )
                    nc.sync.dma_start(
                        out=bcol[:, cb : cb + 1],
                        in_=b_e[0:1, cb * 128 : (cb + 1) * 128].rearrange(
                            "one p -> (one p) 1"
                        ),
                    )
            else:
                wdummy = misc.tile([1, C], F32, tag="wdummy")
                nc.sync.dma_start(out=wdummy, in_=w_e[:, :])
                nc.sync.dma_start(out=wdummy, in_=b_e[:, :])

            def load_seg(src_r, s):
                xt = xin.tile([128, CB, SEG], F32, tag="x")
                for cb in range(CB):
                    nc.sync.dma_start(
                        out=xt[:, cb, :], in_=src_r[cb, :, s * SEG : (s + 1) * SEG]
                    )
                return xt

            def conv_and_square(xt, k):
                """Quarter-seg bf16 convert (ScalarE/GpSimd halves) + square."""
                xbf = bfp.tile([128, CB, QS], BF16, tag="xbf")
                zbf = bfp.tile([128, CB, QS], BF16, tag="zbf")
                xq = xt[:, :, k * QS : (k + 1) * QS]
                hcb = CB // 2
                nc.scalar.copy(out=xbf[:, 0:hcb, :], in_=xq[:, 0:hcb, :])
                nc.vector.tensor_copy(out=xbf[:, hcb:CB, :], in_=xq[:, hcb:CB, :])
                nc.scalar.square(
                    out=zbf.rearrange("p cb t -> p (cb t)"),
                    in_=xbf.rearrange("p cb t -> p (cb t)"),
                )
                return xbf, zbf

            def stats_mms(pool, xbf, zbf, j2, psum=None, start=True, stop=True):
                """s,q sums of quarter-local tile j2 into two [1, TS] PSUM banks."""
                if psum is None:
                    ps_s_t = pool.tile([1, TS], F32, tag="ps_s")
                    ps_q_t = pool.tile([1, TS], F32, tag="ps_q")
                    psum = (ps_s_t, ps_q_t)
                ps_s, ps_q = psum
                for cb in range(CB):
                    nc.tensor.matmul(
                        out=ps_s, lhsT=ones1,
                        rhs=xbf[:, cb, j2 * TS : (j2 + 1) * TS],
                        start=start and cb == 0, stop=stop and cb == CB - 1,
                    )
                for cb in range(CB):
                    nc.tensor.matmul(
                        out=ps_q, lhsT=ones1,
                        rhs=zbf[:, cb, j2 * TS : (j2 + 1) * TS],
                        start=start and cb == 0, stop=stop and cb == CB - 1,
                    )
                return psum

            # ---- prefix reduce phase (totals of xp, flag-gated)
            # No matmuls: ScalarE copy/square with accum_out gives per-partition
            # free-dim sums; one tiny matmul+reduce at the end contracts them.
            NPQ = NSEG * NQS  # 20 prefix quarters
            s_acc = misc.tile([128, NPQ], F32, tag="s_acc")
            q_acc = misc.tile([128, NPQ], F32, tag="q_acc")
            with tc.tile_pool(name="ppre", bufs=1, space="PSUM") as pre_ps:
                for s in range(NSEG):
                    xt = load_seg(xp_r, s)
                    for k in range(NQS):
                        i = s * NQS + k
                        xq = xt[:, :, k * QS : (k + 1) * QS]
                        junk = bfp.tile([128, CB, QS], BF16, tag="xbf")
                        nc.scalar.activation(
                            out=junk, in_=xq,
                            func=mybir.ActivationFunctionType.Square,
                            accum_out=q_acc[:, i : i + 1],
                        )
                        junk2 = bfp.tile([128, CB, QS], BF16, tag="zbf")
                        nc.scalar.activation(
                            out=junk2, in_=xq,
                            func=mybir.ActivationFunctionType.Copy,
                            accum_out=s_acc[:, i : i + 1],
                        )
                tot_s = pre_ps.tile([1, NPQ], F32, tag="tot_s")
                tot_q = pre_ps.tile([1, NPQ], F32, tag="tot_q")
                nc.tensor.matmul(
                    out=tot_s, lhsT=ones1f, rhs=s_acc, start=True, stop=True
                )
                nc.tensor.matmul(
                    out=tot_q, lhsT=ones1f, rhs=q_acc, start=True, stop=True
                )
                sred = misc.tile([1, 1], F32, tag="sred")
                qred = misc.tile([1, 1], F32, tag="qred")
                nc.vector.reduce_sum(out=sred, in_=tot_s, axis=mybir.AxisListType.X)
                nc.vector.reduce_sum(out=qred, in_=tot_q, axis=mybir.AxisListType.X)
                nc.vector.tensor_mul(out=carry_s, in0=sred, in1=flag_t)
                nc.vector.tensor_mul(out=carry_q, in0=qred, in1=flag_t)

            # ---- main phase
            with (
                tc.tile_pool(name="pstat", bufs=2, space="PSUM") as pstat,
                tc.tile_pool(name="pab", bufs=2, space="PSUM") as pab,
            ):
                seg_carries = []
                for s in range(NSEG):
                    xt = load_seg(xc_r, s)
                    srow = rows.tile([1, SEG], F32, tag="srow")
                    qrow = rows.tile([1, SEG], F32, tag="qrow")
                    for k in range(NQS):
                        xbf, zbf = conv_and_square(xt, k)
                        for j2 in range(QS // TS):
                            j = k * (QS // TS) + j2
                            psum = stats_mms(
                                pstat, xbf, zbf, j2, start=True, stop=True
                            )
                            it = s * NTS + j
                            if it == 0:
                                init_s, init_q = carry_s, carry_q
                            elif j == 0:
                                init_s, init_q = seg_carries[s - 1]
                            else:
                                init_s = srow[0:1, j * TS - 1 : j * TS]
                                init_q = qrow[0:1, j * TS - 1 : j * TS]
                            nc.vector.tensor_tensor_scan(
                                out=srow[0:1, j * TS : (j + 1) * TS],
                                data0=psum[0], data1=zrow, initial=init_s,
                                op0=mybir.AluOpType.add, op1=mybir.AluOpType.bypass,
                            )
                            nc.vector.tensor_tensor_scan(
                                out=qrow[0:1, j * TS : (j + 1) * TS],
                                data0=psum[1], data1=zrow, initial=init_q,
                                op0=mybir.AluOpType.add, op1=mybir.AluOpType.bypass,
                            )

                    segc_s = segc.tile([1, 1], F32, tag="segc_s")
                    segc_q = segc.tile([1, 1], F32, tag="segc_q")
                    nc.vector.tensor_copy(out=segc_s, in_=srow[0:1, SEG - 1 : SEG])
                    nc.vector.tensor_copy(out=segc_q, in_=qrow[0:1, SEG - 1 : SEG])
                    seg_carries.append((segc_s, segc_q))

                    # ---- segment finalize in t-major [128, F]
                    d_sq = dram.tile([2 * SEG], F32, tag="d_sq")
                    d_sq2 = d_sq.rearrange("(s t) -> s t", s=2)
                    nc.sync.dma_start(out=d_sq2[0:1, :], in_=srow)
                    nc.sync.dma_start(out=d_sq2[1:2, :], in_=qrow)
                    tm2 = tmaj.tile([128, 2, F], F32, tag="tm2")
                    nc.sync.dma_start(
                        out=tm2, in_=d_sq.rearrange("(s p f) -> p s f", s=2, p=128)
                    )
                    s_tm = tm2[:, 0, :]
                    q_tm = tm2[:, 1, :]
                    invp_s = invp_t[:, s * F : (s + 1) * F]
                    invm_s = invm_t[:, s * F : (s + 1) * F]
                    nmean = tmaj.tile([128, F], F32, tag="nmean")
                    nc.vector.tensor_mul(out=nmean, in0=s_tm, in1=invm_s)  # -mean
                    e2 = tmaj.tile([128, F], F32, tag="e2")
                    nc.vector.tensor_mul(out=e2, in0=q_tm, in1=invp_s)
                    msq = tmaj.tile([128, F], F32, tag="msq")
                    nc.vector.tensor_mul(out=msq, in0=nmean, in1=nmean)
                    var = tmaj.tile([128, F], F32, tag="var")
                    nc.vector.tensor_sub(out=var, in0=e2, in1=msq)
                    nc.vector.tensor_scalar_max(out=var, in0=var, scalar1=0.0)
                    sd = tmaj.tile([128, F], F32, tag="sd")
                    nc.scalar.activation(
                        out=sd, in_=var, func=mybir.ActivationFunctionType.Sqrt,
                        bias=eps_t, scale=1.0,
                    )
                    tmo = tmaj.tile([128, 2, F], F32R, tag="tmo")
                    with nc.allow_low_precision(
                        reason="f32r rounding feeds PE broadcast matmuls"
                    ):
                        nc.vector.reciprocal(out=tmo[:, 0, :], in_=sd)
                        nc.vector.tensor_mul(
                            out=tmo[:, 1, :], in0=nmean, in1=tmo[:, 0, :]
                        )
                    d_ab = dram.tile([2 * SEG], F32R, tag="d_ab")
                    nc.sync.dma_start(
                        out=d_ab.rearrange("(s p f) -> p s f", s=2, p=128), in_=tmo
                    )
                    invrow = abrow.tile([1, SEG], F32R, tag="invrow")
                    nminvrow = abrow.tile([1, SEG], F32R, tag="nminvrow")
                    d_ab2 = d_ab.rearrange("(s t) -> s t", s=2)
                    nc.sync.dma_start(out=invrow, in_=d_ab2[0:1, :])
                    nc.sync.dma_start(out=nminvrow, in_=d_ab2[1:2, :])

                    # ---- normalize segment in place: y = x*A + B
                    for j in range(NTN):
                        ps_a = pab.tile([128, TN], F32, tag="ps_a")
                        ps_b = pab.tile([128, TN], F32, tag="ps_b")
                        nc.tensor.matmul(
                            out=ps_a, lhsT=ones_r,
                            rhs=invrow[0:1, j * TN : (j + 1) * TN],
                            start=True, stop=True,
                        )
                        nc.tensor.matmul(
                            out=ps_b, lhsT=ones_r,
                            rhs=nminvrow[0:1, j * TN : (j + 1) * TN],
                            start=True, stop=True,
                        )
                        rep_a = bass.AP(
                            tensor=ps_a.tensor, offset=ps_a.offset,
                            ap=[ps_a.ap[0], [0, CB], ps_a.ap[1]],
                        )
                        rep_b = bass.AP(
                            tensor=ps_b.tensor, offset=ps_b.offset,
                            ap=[ps_b.ap[0], [0, CB], ps_b.ap[1]],
                        )
                        xs = xt[:, :, j * TN : (j + 1) * TN]
                        nc.vector.tensor_mul(out=xs, in0=xs, in1=rep_a)
                        nc.vector.tensor_add(out=xs, in0=xs, in1=rep_b)
                        if wb_general:
                            for cb in range(CB):
                                nc.scalar.activation(
                                    out=xs[:, cb, :], in_=xs[:, cb, :],
                                    func=mybir.ActivationFunctionType.Copy,
                                    bias=0.0, scale=wcol[:, cb : cb + 1],
                                )
                                nc.vector.tensor_scalar_add(
                                    out=xs[:, cb, :], in0=xs[:, cb, :],
                                    scalar1=bcol[:, cb : cb + 1],
                                )
                    for cb in range(CB):
                        nc.sync.dma_start(
                            out=y_r[cb, :, s * SEG : (s + 1) * SEG], in_=xt[:, cb, :]
                        )

    nc.finalize()
    return nc


def _get_kernel(wb_general: bool):
    if wb_general not in _CACHE:
        _CACHE[wb_general] = _build(wb_general)
    return _CACHE[wb_general]


def _make_in_maps(x, weight, bias):
    wb_general = not (np.all(weight == 1.0) and np.all(bias == 0.0))
    w_row = np.ascontiguousarray(weight.reshape(1, C).astype(np.float32))
    b_row = np.ascontiguousarray(bias.reshape(1, C).astype(np.float32))
    in_maps = []
    for core in range(NCORES):
        b_idx, h = core // 2, core % 2
        xc = np.ascontiguousarray(x[b_idx, :, h * TH : (h + 1) * TH])
        xp = np.ascontiguousarray(x[b_idx, :, 0:TH]) if h == 1 else xc
        flag = np.full((2, 1), float(h), np.float32)
        # invn[p, s*F + f] = 1 / (C * (h*TH + s*SEG + p*F + f + 1))
        t_local = (
            np.arange(NSEG)[:, None, None] * SEG
            + np.arange(128)[None, :, None] * F
            + np.arange(F)[None, None, :]
        )
        t_global = h * TH + t_local  # [NSEG, 128, F]
        invn = (1.0 / (C * (t_global.astype(np.float64) + 1.0))).astype(np.float32)
        invn = np.ascontiguousarray(invn.transpose(1, 0, 2).reshape(128, NSEG * F))
        in_maps.append(
            {
                "xc": xc, "xp": xp, "flag": flag,
                "invp": invn, "invm": np.ascontiguousarray(-invn),
                "w": w_row, "b": b_row,
            }
        )
    return in_maps, wb_general


def kernel(x, weight, bias, _trace=False, _tmpdir=None):
    x = np.asarray(x, np.float32)
    weight = np.asarray(weight, np.float32)
    bias = np.asarray(bias, np.float32)
    in_maps, wb_general = _make_in_maps(x, weight, bias)
    nc = _get_kernel(wb_general)
    res = run_bass_kernel_spmd(
        nc, in_maps, list(range(NCORES)), trace=_trace, tmpdir=_tmpdir
    )
    y = np.empty((B, C, T), np.float32)
    for core in range(NCORES):
        b_idx, h = core // 2, core % 2
        y[b_idx, :, h * TH : (h + 1) * TH] = res.results[core]["y"]
    if _trace:
        return y, res
    return y


# revision 19
# speedup vs baseline: 1.0471x; 1.0471x over previous
"""Cumulative LayerNorm (B=4, C=512, T=32000) on 8 Trainium2 NeuronCores.

Sharding: core j handles batch b = j//2, T-half h = j%2.
Cumulative stats over (C, 0..t) need a carry from the first T-half; each
h=1 core recomputes it with a prefix reduce over that half (no cross-core
comm). SPMD symmetry: h=0 cores run the same prefix pass on their own
data and multiply the carry by flag=0.

Per-core pipeline, one streaming pass per 3200-t segment:
  - 4 big DMAs (1.6MB, 12.8KB/partition lines) load the segment
  - fp32->bf16 converts (split ScalarE/GpSimd), bf16 squares (ScalarE)
  - per-t channel sums: M=2 matmuls (lhsT e0/e1 put s on partition 0,
    q on partition 1 of one PSUM bank; LDWEIGHTS ~free at 2 columns)
  - running cumsum: tensor_tensor_scan on the PSUM rows, chained across
    all tiles by `initial` APs (VectorE)
  - per segment: one DMA reshapes [2,3200] rows to t-major [128,50];
    stats finalized there (mean/var/recip-sqrt, tiny [128,25] ops);
    f32r-rounded inv and -mean*inv reshaped back to rows
  - A=ones x inv, B=ones x (-mean*inv) K=1 f32r matmuls -> PSUM [128,400]
  - y = x*A + B in place on the x segment: 2 TT ops per 400-block with
    cb-repeat PSUM APs (VectorE)
  - 4 big DMAs store the segment
"""
import numpy as np

import concourse.bass as bass
import concourse.bacc as bacc
import concourse.tile as tile
from concourse import mybir
from concourse.bass_utils import run_bass_kernel_spmd

F32 = mybir.dt.float32
F32R = mybir.dt.float32r
BF16 = mybir.dt.bfloat16

B, C, T = 4, 512, 32000
NCORES = 8
TH = T // 2          # 16000 per core
CB = C // 128        # 4 channel blocks
SEG = 3200           # segment
F = SEG // 128       # 25 (t-major free dim per segment)
NSEG = TH // SEG     # 5
TS = 400             # stats matmul tile (N=400 moving cols)
NTS = SEG // TS      # 8
TN = 400             # normalize block (A/B psum [128, 400])
NTN = SEG // TN      # 8
QS = 800             # convert/square op granularity (quarter segment)
NQS = SEG // QS      # 4
EPS = 1e-08

_CACHE = {}


def _build(wb_general: bool):
    nc = bacc.Bacc()

    xc_e = nc.declare_dram_parameter("xc", [C, TH], F32, isOutput=False)
    xp_e = nc.declare_dram_parameter("xp", [C, TH], F32, isOutput=False)
    flag_e = nc.declare_dram_parameter("flag", [1, 1], F32, isOutput=False)
    invp_e = nc.declare_dram_parameter("invp", [128, F * NSEG], F32, isOutput=False)
    invm_e = nc.declare_dram_parameter("invm", [128, F * NSEG], F32, isOutput=False)
    w_e = nc.declare_dram_parameter("w", [1, C], F32, isOutput=False)
    b_e = nc.declare_dram_parameter("b", [1, C], F32, isOutput=False)
    y_e = nc.declare_dram_parameter("y", [C, TH], F32, isOutput=True)

    xc_r = xc_e.rearrange("(cb p) t -> cb p t", p=128)
    xp_r = xp_e.rearrange("(cb p) t -> cb p t", p=128)
    y_r = y_e.rearrange("(cb p) t -> cb p t", p=128)

    with tile.TileContext(nc) as tc:
        with (
            tc.tile_pool(name="misc", bufs=1) as misc,
            tc.tile_pool(name="xin", bufs=2) as xin,
            tc.tile_pool(name="bfp", bufs=2) as bfp,
            tc.tile_pool(name="rows", bufs=1) as rows,
            tc.tile_pool(name="segc", bufs=2) as segc,
            tc.tile_pool(name="abrow", bufs=1) as abrow,
            tc.tile_pool(name="tmaj", bufs=2) as tmaj,
            tc.tile_pool(name="dram", bufs=3, space="DRAM") as dram,
        ):
            # ---- constants
            # M=1 ones column: stats land on partition 0, LDWEIGHTS ~free
            ones1 = misc.tile([128, 1], BF16, tag="ones1")
            nc.vector.memset(ones1, 1.0)
            ones_f = misc.tile([1, 128], F32, tag="ones_f")
            nc.vector.memset(ones_f, 1.0)
            ones_r = misc.tile([1, 128], F32R, tag="ones_r")
            nc.scalar.copy(out=ones_r, in_=ones_f)
            eps_t = misc.tile([128, 1], F32, tag="eps_t")
            nc.vector.memset(eps_t, EPS)
            zrow = misc.tile([1, TS], F32, tag="zrow")
            nc.vector.memset(zrow, 0.0)
            flag_t = misc.tile([1, 1], F32, tag="flag_t")
            nc.sync.dma_start(out=flag_t, in_=flag_e[:, :])
            invp_t = misc.tile([128, F * NSEG], F32, tag="invp_t")
            nc.sync.dma_start(out=invp_t, in_=invp_e[:, :])
            invm_t = misc.tile([128, F * NSEG], F32, tag="invm_t")
            nc.sync.dma_start(out=invm_t, in_=invm_e[:, :])
            carry_s = misc.tile([1, 1], F32, tag="carry_s")
            carry_q = misc.tile([1, 1], F32, tag="carry_q")
            if wb_general:
                wcol = misc.tile([128, CB], F32, tag="wcol")
                bcol = misc.tile([128, CB], F32, tag="bcol")
                for cb in range(CB):
                    nc.sync.dma_start(
                        out=wcol[:, cb : cb + 1],
                        in_=w_e[0:1, cb * 128 : (cb + 1) * 128].rearrange(
                            "one p -> (one p) 1"
                        ),
                    )
                    nc.sync.dma_start(
                        out=bcol[:, cb : cb + 1],
                        in_=b_e[0:1, cb * 128 : (cb + 1) * 128].rearrange(
                            "one p -> (one p) 1"
                        ),
                    )
            else:
                wdummy = misc.tile([1, C], F32, tag="wdummy")
                nc.sync.dma_start(out=wdummy, in_=w_e[:, :])
                nc.sync.dma_start(out=wdummy, in_=b_e[:, :])

            def load_seg(src_r, s):
                xt = xin.tile([128, CB, SEG], F32, tag="x")
                for cb in range(CB):
                    nc.sync.dma_start(
                        out=xt[:, cb, :], in_=src_r[cb, :, s * SEG : (s + 1) * SEG]
                    )
                return xt

            def conv_and_square(xt, k):
                """Quarter-seg bf16 convert (ScalarE/GpSimd halves) + square."""
                xbf = bfp.tile([128, CB, QS], BF16, tag="xbf")
                zbf = bfp.tile([128, CB, QS], BF16, tag="zbf")
                xq = xt[:, :, k * QS : (k + 1) * QS]
                nc.scalar.copy(out=xbf[:, 0:3, :], in_=xq[:, 0:3, :])
                nc.vector.tensor_copy(out=xbf[:, 3:CB, :], in_=xq[:, 3:CB, :])
                nc.scalar.square(
                    out=zbf.rearrange("p cb t -> p (cb t)"),
                    in_=xbf.rearrange("p cb t -> p (cb t)"),
                )
                return xbf, zbf

            def stats_mms(pool, xbf, zbf, j2, psum=None, start=True, stop=True):
                """s,q sums of quarter-local tile j2 into two [1, TS] PSUM banks."""
                if psum is None:
                    ps_s_t = pool.tile([1, TS], F32, tag="ps_s")
                    ps_q_t = pool.tile([1, TS], F32, tag="ps_q")
                    psum = (ps_s_t, ps_q_t)
                ps_s, ps_q = psum
                for cb in range(CB):
                    nc.tensor.matmul(
                        out=ps_s, lhsT=ones1,
                        rhs=xbf[:, cb, j2 * TS : (j2 + 1) * TS],
                        start=start and cb == 0, stop=stop and cb == CB - 1,
                    )
                for cb in range(CB):
                    nc.tensor.matmul(
                        out=ps_q, lhsT=ones1,
                        rhs=zbf[:, cb, j2 * TS : (j2 + 1) * TS],
                        start=start and cb == 0, stop=stop and cb == CB - 1,
                    )
                return psum

            # ---- prefix reduce phase (totals of xp, flag-gated)
            with tc.tile_pool(name="ppre", bufs=1, space="PSUM") as pre_ps:
                tot_s = pre_ps.tile([1, TS], F32, tag="tot_s")
                tot_q = pre_ps.tile([1, TS], F32, tag="tot_q")
                tot = (tot_s, tot_q)
                for s in range(NSEG):
                    xt = load_seg(xp_r, s)
                    for k in range(NQS):
                        xbf, zbf = conv_and_square(xt, k)
                        for j2 in range(QS // TS):
                            stats_mms(
                                pre_ps, xbf, zbf, j2, psum=tot,
                                start=(s == 0 and k == 0 and j2 == 0),
                                stop=(
                                    s == NSEG - 1
                                    and k == NQS - 1
                                    and j2 == QS // TS - 1
                                ),
                            )
                sred = misc.tile([1, 1], F32, tag="sred")
                qred = misc.tile([1, 1], F32, tag="qred")
                nc.vector.reduce_sum(out=sred, in_=tot[0], axis=mybir.AxisListType.X)
                nc.vector.reduce_sum(out=qred, in_=tot[1], axis=mybir.AxisListType.X)
                nc.vector.tensor_mul(out=carry_s, in0=sred, in1=flag_t)
                nc.vector.tensor_mul(out=carry_q, in0=qred, in1=flag_t)

            # ---- main phase
            with (
                tc.tile_pool(name="pstat", bufs=2, space="PSUM") as pstat,
                tc.tile_pool(name="pab", bufs=2, space="PSUM") as pab,
            ):
                seg_carries = []
                for s in range(NSEG):
                    xt = load_seg(xc_r, s)
                    srow = rows.tile([1, SEG], F32, tag="srow")
                    qrow = rows.tile([1, SEG], F32, tag="qrow")
                    for k in range(NQS):
                        xbf, zbf = conv_and_square(xt, k)
                        for j2 in range(QS // TS):
                            j = k * (QS // TS) + j2
                            psum = stats_mms(
                                pstat, xbf, zbf, j2, start=True, stop=True
                            )
                            it = s * NTS + j
                            if it == 0:
                                init_s, init_q = carry_s, carry_q
                            elif j == 0:
                                init_s, init_q = seg_carries[s - 1]
                            else:
                                init_s = srow[0:1, j * TS - 1 : j * TS]
                                init_q = qrow[0:1, j * TS - 1 : j * TS]
                            nc.vector.tensor_tensor_scan(
                                out=srow[0:1, j * TS : (j + 1) * TS],
                                data0=psum[0], data1=zrow, initial=init_s,
                                op0=mybir.AluOpType.add, op1=mybir.AluOpType.bypass,
                            )
                            nc.vector.tensor_tensor_scan(
                                out=qrow[0:1, j * TS : (j + 1) * TS],
                                data0=psum[1], data1=zrow, initial=init_q,
                                op0=mybir.AluOpType.add, op1=mybir.AluOpType.bypass,
                            )

                    segc_s = segc.tile([1, 1], F32, tag="segc_s")
                    segc_q = segc.tile([1, 1], F32, tag="segc_q")
                    nc.vector.tensor_copy(out=segc_s, in_=srow[0:1, SEG - 1 : SEG])
                    nc.vector.tensor_copy(out=segc_q, in_=qrow[0:1, SEG - 1 : SEG])
                    seg_carries.append((segc_s, segc_q))

                    # ---- segment finalize in t-major [128, F]
                    d_sq = dram.tile([2 * SEG], F32, tag="d_sq")
                    d_sq2 = d_sq.rearrange("(s t) -> s t", s=2)
                    nc.sync.dma_start(out=d_sq2[0:1, :], in_=srow)
                    nc.sync.dma_start(out=d_sq2[1:2, :], in_=qrow)
                    tm2 = tmaj.tile([128, 2, F], F32, tag="tm2")
                    nc.sync.dma_start(
                        out=tm2, in_=d_sq.rearrange("(s p f) -> p s f", s=2, p=128)
                    )
                    s_tm = tm2[:, 0, :]
                    q_tm = tm2[:, 1, :]
                    invp_s = invp_t[:, s * F : (s + 1) * F]
                    invm_s = invm_t[:, s * F : (s + 1) * F]
                    nmean = tmaj.tile([128, F], F32, tag="nmean")
                    nc.vector.tensor_mul(out=nmean, in0=s_tm, in1=invm_s)  # -mean
                    e2 = tmaj.tile([128, F], F32, tag="e2")
                    nc.vector.tensor_mul(out=e2, in0=q_tm, in1=invp_s)
                    msq = tmaj.tile([128, F], F32, tag="msq")
                    nc.vector.tensor_mul(out=msq, in0=nmean, in1=nmean)
                    var = tmaj.tile([128, F], F32, tag="var")
                    nc.vector.tensor_sub(out=var, in0=e2, in1=msq)
                    nc.vector.tensor_scalar_max(out=var, in0=var, scalar1=0.0)
                    sd = tmaj.tile([128, F], F32, tag="sd")
                    nc.scalar.activation(
                        out=sd, in_=var, func=mybir.ActivationFunctionType.Sqrt,
                        bias=eps_t, scale=1.0,
                    )
                    tmo = tmaj.tile([128, 2, F], F32R, tag="tmo")
                    with nc.allow_low_precision(
                        reason="f32r rounding feeds PE broadcast matmuls"
                    ):
                        nc.vector.reciprocal(out=tmo[:, 0, :], in_=sd)
                        nc.vector.tensor_mul(
                            out=tmo[:, 1, :], in0=nmean, in1=tmo[:, 0, :]
                        )
                    d_ab = dram.tile([2 * SEG], F32R, tag="d_ab")
                    nc.sync.dma_start(
                        out=d_ab.rearrange("(s p f) -> p s f", s=2, p=128), in_=tmo
                    )
                    invrow = abrow.tile([1, SEG], F32R, tag="invrow")
                    nminvrow = abrow.tile([1, SEG], F32R, tag="nminvrow")
                    d_ab2 = d_ab.rearrange("(s t) -> s t", s=2)
                    nc.sync.dma_start(out=invrow, in_=d_ab2[0:1, :])
                    nc.sync.dma_start(out=nminvrow, in_=d_ab2[1:2, :])

                    # ---- normalize segment in place: y = x*A + B
                    for j in range(NTN):
                        ps_a = pab.tile([128, TN], F32, tag="ps_a")
                        ps_b = pab.tile([128, TN], F32, tag="ps_b")
                        nc.tensor.matmul(
                            out=ps_a, lhsT=ones_r,
                            rhs=invrow[0:1, j * TN : (j + 1) * TN],
                            start=True, stop=True,
                        )
                        nc.tensor.matmul(
                            out=ps_b, lhsT=ones_r,
                            rhs=nminvrow[0:1, j * TN : (j + 1) * TN],
                            start=True, stop=True,
                        )
                        rep_a = bass.AP(
                            tensor=ps_a.tensor, offset=ps_a.offset,
                            ap=[ps_a.ap[0], [0, CB], ps_a.ap[1]],
                        )
                        rep_b = bass.AP(
                            tensor=ps_b.tensor, offset=ps_b.offset,
                            ap=[ps_b.ap[0], [0, CB], ps_b.ap[1]],
                        )
                        xs = xt[:, :, j * TN : (j + 1) * TN]
                        nc.vector.tensor_mul(out=xs, in0=xs, in1=rep_a)
                        nc.vector.tensor_add(out=xs, in0=xs, in1=rep_b)
                        if wb_general:
                            for cb in range(CB):
                                nc.scalar.activation(
                                    out=xs[:, cb, :], in_=xs[:, cb, :],
                                    func=mybir.ActivationFunctionType.Copy,
                                    bias=0.0, scale=wcol[:, cb : cb + 1],
                                )
                                nc.vector.tensor_scalar_add(
                                    out=xs[:, cb, :], in0=xs[:, cb, :],
                                    scalar1=bcol[:, cb : cb + 1],
                                )
                    for cb in range(CB):
                        nc.sync.dma_start(
                            out=y_r[cb, :, s * SEG : (s + 1) * SEG], in_=xt[:, cb, :]
                        )

    nc.finalize()
    return nc


def _get_kernel(wb_general: bool):
    if wb_general not in _CACHE:
        _CACHE[wb_general] = _build(wb_general)
    return _CACHE[wb_general]


def _make_in_maps(x, weight, bias):
    wb_general = not (np.all(weight == 1.0) and np.all(bias == 0.0))
    w_row = np.ascontiguousarray(weight.reshape(1, C).astype(np.float32))
    b_row = np.ascontiguousarray(bias.reshape(1, C).astype(np.float32))
    in_maps = []
    for core in range(NCORES):
        b_idx, h = core // 2, core % 2
        xc = np.ascontiguousarray(x[b_idx, :, h * TH : (h + 1) * TH])
        xp = np.ascontiguousarray(x[b_idx, :, 0:TH]) if h == 1 else xc
        flag = np.full((2, 1), float(h), np.float32)
        # invn[p, s*F + f] = 1 / (C * (h*TH + s*SEG + p*F + f + 1))
        t_local = (
            np.arange(NSEG)[:, None, None] * SEG
            + np.arange(128)[None, :, None] * F
            + np.arange(F)[None, None, :]
        )
        t_global = h * TH + t_local  # [NSEG, 128, F]
        invn = (1.0 / (C * (t_global.astype(np.float64) + 1.0))).astype(np.float32)
        invn = np.ascontiguousarray(invn.transpose(1, 0, 2).reshape(128, NSEG * F))
        in_maps.append(
            {
                "xc": xc, "xp": xp, "flag": flag,
                "invp": invn, "invm": np.ascontiguousarray(-invn),
                "w": w_row, "b": b_row,
            }
        )
    return in_maps, wb_general


def kernel(x, weight, bias, _trace=False, _tmpdir=None):
    x = np.asarray(x, np.float32)
    weight = np.asarray(weight, np.float32)
    bias = np.asarray(bias, np.float32)
    in_maps, wb_general = _make_in_maps(x, weight, bias)
    nc = _get_kernel(wb_general)
    res = run_bass_kernel_spmd(
        nc, in_maps, list(range(NCORES)), trace=_trace, tmpdir=_tmpdir
    )
    y = np.empty((B, C, T), np.float32)
    for core in range(NCORES):
        b_idx, h = core // 2, core % 2
        y[b_idx, :, h * TH : (h + 1) * TH] = res.results[core]["y"]
    if _trace:
        return y, res
    return y


# revision 20
# speedup vs baseline: 1.0798x; 1.0312x over previous
"""Cumulative LayerNorm (B=4, C=512, T=32000) on 8 Trainium2 NeuronCores.

Sharding: core j handles batch b = j//2, T-half h = j%2.
Cumulative stats over (C, 0..t) need a carry from the first T-half; each
h=1 core recomputes it with a prefix reduce over that half (no cross-core
comm). SPMD symmetry: h=0 cores run the same prefix pass on their own
data and multiply the carry by flag=0.

Per-core pipeline, one streaming pass per 3200-t segment:
  - 4 big DMAs (1.6MB, 12.8KB/partition lines) load the segment
  - fp32->bf16 converts (split ScalarE/GpSimd), bf16 squares (ScalarE)
  - per-t channel sums: M=2 matmuls (lhsT e0/e1 put s on partition 0,
    q on partition 1 of one PSUM bank; LDWEIGHTS ~free at 2 columns)
  - running cumsum: tensor_tensor_scan on the PSUM rows, chained across
    all tiles by `initial` APs (VectorE)
  - per segment: one DMA reshapes [2,3200] rows to t-major [128,50];
    stats finalized there (mean/var/recip-sqrt, tiny [128,25] ops);
    f32r-rounded inv and -mean*inv reshaped back to rows
  - A=ones x inv, B=ones x (-mean*inv) K=1 f32r matmuls -> PSUM [128,400]
  - y = x*A + B in place on the x segment: 2 TT ops per 400-block with
    cb-repeat PSUM APs (VectorE)
  - 4 big DMAs store the segment
"""
import numpy as np

import concourse.bass as bass
import concourse.bacc as bacc
import concourse.tile as tile
from concourse import mybir
from concourse.bass_utils import run_bass_kernel_spmd

F32 = mybir.dt.float32
F32R = mybir.dt.float32r
BF16 = mybir.dt.bfloat16

B, C, T = 4, 512, 32000
NCORES = 8
TH = T // 2          # 16000 per core
CB = C // 128        # 4 channel blocks
SEG = 3200           # segment
F = SEG // 128       # 25 (t-major free dim per segment)
NSEG = TH // SEG     # 5
TS = 400             # stats matmul tile (N=400 moving cols)
NTS = SEG // TS      # 8
TN = 400             # normalize block (A/B psum [128, 400])
NTN = SEG // TN      # 8
QS = 800             # convert/square op granularity (quarter segment)
NQS = SEG // QS      # 4
EPS = 1e-08

_CACHE = {}


def _build(wb_general: bool):
    nc = bacc.Bacc()

    xc_e = nc.declare_dram_parameter("xc", [C, TH], F32, isOutput=False)
    xp_e = nc.declare_dram_parameter("xp", [C, TH], F32, isOutput=False)
    flag_e = nc.declare_dram_parameter("flag", [1, 1], F32, isOutput=False)
    invp_e = nc.declare_dram_parameter("invp", [128, F * NSEG], F32, isOutput=False)
    invm_e = nc.declare_dram_parameter("invm", [128, F * NSEG], F32, isOutput=False)
    w_e = nc.declare_dram_parameter("w", [1, C], F32, isOutput=False)
    b_e = nc.declare_dram_parameter("b", [1, C], F32, isOutput=False)
    y_e = nc.declare_dram_parameter("y", [C, TH], F32, isOutput=True)

    xc_r = xc_e.rearrange("(cb p) t -> cb p t", p=128)
    xp_r = xp_e.rearrange("(cb p) t -> cb p t", p=128)
    y_r = y_e.rearrange("(cb p) t -> cb p t", p=128)

    with tile.TileContext(nc) as tc:
        with (
            tc.tile_pool(name="misc", bufs=1) as misc,
            tc.tile_pool(name="xin", bufs=2) as xin,
            tc.tile_pool(name="bfp", bufs=2) as bfp,
            tc.tile_pool(name="rows", bufs=1) as rows,
            tc.tile_pool(name="segc", bufs=2) as segc,
            tc.tile_pool(name="abrow", bufs=1) as abrow,
            tc.tile_pool(name="tmaj", bufs=2) as tmaj,
            tc.tile_pool(name="dram", bufs=3, space="DRAM") as dram,
        ):
            # ---- constants
            # M=1 ones column: stats land on partition 0, LDWEIGHTS ~free
            ones1 = misc.tile([128, 1], BF16, tag="ones1")
            nc.vector.memset(ones1, 1.0)
            ones1f = misc.tile([128, 1], F32, tag="ones1f")
            nc.vector.memset(ones1f, 1.0)
            ones_f = misc.tile([1, 128], F32, tag="ones_f")
            nc.vector.memset(ones_f, 1.0)
            ones_r = misc.tile([1, 128], F32R, tag="ones_r")
            nc.scalar.copy(out=ones_r, in_=ones_f)
            eps_t = misc.tile([128, 1], F32, tag="eps_t")
            nc.vector.memset(eps_t, EPS)
            zrow = misc.tile([1, TS], F32, tag="zrow")
            nc.vector.memset(zrow, 0.0)
            flag_t = misc.tile([1, 1], F32, tag="flag_t")
            nc.sync.dma_start(out=flag_t, in_=flag_e[:, :])
            invp_t = misc.tile([128, F * NSEG], F32, tag="invp_t")
            nc.sync.dma_start(out=invp_t, in_=invp_e[:, :])
            invm_t = misc.tile([128, F * NSEG], F32, tag="invm_t")
            nc.sync.dma_start(out=invm_t, in_=invm_e[:, :])
            carry_s = misc.tile([1, 1], F32, tag="carry_s")
            carry_q = misc.tile([1, 1], F32, tag="carry_q")
            if wb_general:
                wcol = misc.tile([128, CB], F32, tag="wcol")
                bcol = misc.tile([128, CB], F32, tag="bcol")
                for cb in range(CB):
                    nc.sync.dma_start(
                        out=wcol[:, cb : cb + 1],
                        in_=w_e[0:1, cb * 128 : (cb + 1) * 128].rearrange(
                            "one p -> (one p) 1"
                        ),
                    )
                    nc.sync.dma_start(
                        out=bcol[:, cb : cb + 1],
                        in_=b_e[0:1, cb * 128 : (cb + 1) * 128].rearrange(
                            "one p -> (one p) 1"
                        ),
                    )
            else:
                wdummy = misc.tile([1, C], F32, tag="wdummy")
                nc.sync.dma_start(out=wdummy, in_=w_e[:, :])
                nc.sync.dma_start(out=wdummy, in_=b_e[:, :])

            def load_seg(src_r, s):
                xt = xin.tile([128, CB, SEG], F32, tag="x")
                for cb in range(CB):
                    nc.sync.dma_start(
                        out=xt[:, cb, :], in_=src_r[cb, :, s * SEG : (s + 1) * SEG]
                    )
                return xt

            def conv_and_square(xt, k, q_accum=None):
                """Quarter-seg bf16 convert (ScalarE/VectorE split) + square."""
                xbf = bfp.tile([128, CB, QS], BF16, tag="xbf")
                zbf = bfp.tile([128, CB, QS], BF16, tag="zbf")
                xq = xt[:, :, k * QS : (k + 1) * QS]
                nc.scalar.copy(out=xbf[:, 0:3, :], in_=xq[:, 0:3, :])
                nc.vector.tensor_copy(out=xbf[:, 3:CB, :], in_=xq[:, 3:CB, :])
                nc.scalar.activation(
                    out=zbf.rearrange("p cb t -> p (cb t)"),
                    in_=xbf.rearrange("p cb t -> p (cb t)"),
                    func=mybir.ActivationFunctionType.Square,
                    accum_out=q_accum,
                )
                return xbf, zbf

            def stats_mms(pool, xbf, zbf, j2, psum=None, start=True, stop=True):
                """s,q sums of quarter-local tile j2 into two [1, TS] PSUM banks."""
                if psum is None:
                    ps_s_t = pool.tile([1, TS], F32, tag="ps_s")
                    ps_q_t = pool.tile([1, TS], F32, tag="ps_q")
                    psum = (ps_s_t, ps_q_t)
                ps_s, ps_q = psum
                for cb in range(CB):
                    nc.tensor.matmul(
                        out=ps_s, lhsT=ones1,
                        rhs=xbf[:, cb, j2 * TS : (j2 + 1) * TS],
                        start=start and cb == 0, stop=stop and cb == CB - 1,
                    )
                if ps_q is not None:
                    for cb in range(CB):
                        nc.tensor.matmul(
                            out=ps_q, lhsT=ones1,
                            rhs=zbf[:, cb, j2 * TS : (j2 + 1) * TS],
                            start=start and cb == 0, stop=stop and cb == CB - 1,
                        )
                return psum

            # ---- prefix reduce phase (totals of xp, flag-gated)
            NPQ = NSEG * NQS  # 20 prefix quarters
            q_acc = misc.tile([128, NPQ], F32, tag="q_acc")
            with tc.tile_pool(name="ppre", bufs=1, space="PSUM") as pre_ps:
                tot_s = pre_ps.tile([1, TS], F32, tag="tot_s")
                tot = (tot_s, None)
                for s in range(NSEG):
                    xt = load_seg(xp_r, s)
                    for k in range(NQS):
                        i = s * NQS + k
                        xbf, zbf = conv_and_square(
                            xt, k, q_accum=q_acc[:, i : i + 1]
                        )
                        for j2 in range(QS // TS):
                            stats_mms(
                                pre_ps, xbf, zbf, j2, psum=tot,
                                start=(s == 0 and k == 0 and j2 == 0),
                                stop=(
                                    s == NSEG - 1
                                    and k == NQS - 1
                                    and j2 == QS // TS - 1
                                ),
                            )
                tot_q = pre_ps.tile([1, NPQ], F32, tag="tot_q")
                nc.tensor.matmul(
                    out=tot_q, lhsT=ones1f, rhs=q_acc, start=True, stop=True
                )
                sred = misc.tile([1, 1], F32, tag="sred")
                qred = misc.tile([1, 1], F32, tag="qred")
                nc.vector.reduce_sum(out=sred, in_=tot[0], axis=mybir.AxisListType.X)
                nc.vector.reduce_sum(out=qred, in_=tot_q, axis=mybir.AxisListType.X)
                nc.vector.tensor_mul(out=carry_s, in0=sred, in1=flag_t)
                nc.vector.tensor_mul(out=carry_q, in0=qred, in1=flag_t)

            # ---- main phase
            with (
                tc.tile_pool(name="pstat", bufs=2, space="PSUM") as pstat,
                tc.tile_pool(name="pab", bufs=2, space="PSUM") as pab,
            ):
                seg_carries = []
                for s in range(NSEG):
                    xt = load_seg(xc_r, s)
                    srow = rows.tile([1, SEG], F32, tag="srow")
                    qrow = rows.tile([1, SEG], F32, tag="qrow")
                    for k in range(NQS):
                        xbf, zbf = conv_and_square(xt, k)
                        for j2 in range(QS // TS):
                            j = k * (QS // TS) + j2
                            psum = stats_mms(
                                pstat, xbf, zbf, j2, start=True, stop=True
                            )
                            it = s * NTS + j
                            if it == 0:
                                init_s, init_q = carry_s, carry_q
                            elif j == 0:
                                init_s, init_q = seg_carries[s - 1]
                            else:
                                init_s = srow[0:1, j * TS - 1 : j * TS]
                                init_q = qrow[0:1, j * TS - 1 : j * TS]
                            nc.vector.tensor_tensor_scan(
                                out=srow[0:1, j * TS : (j + 1) * TS],
                                data0=psum[0], data1=zrow, initial=init_s,
                                op0=mybir.AluOpType.add, op1=mybir.AluOpType.bypass,
                            )
                            nc.vector.tensor_tensor_scan(
                                out=qrow[0:1, j * TS : (j + 1) * TS],
                                data0=psum[1], data1=zrow, initial=init_q,
                                op0=mybir.AluOpType.add, op1=mybir.AluOpType.bypass,
                            )

                    segc_s = segc.tile([1, 1], F32, tag="segc_s")
                    segc_q = segc.tile([1, 1], F32, tag="segc_q")
                    nc.vector.tensor_copy(out=segc_s, in_=srow[0:1, SEG - 1 : SEG])
                    nc.vector.tensor_copy(out=segc_q, in_=qrow[0:1, SEG - 1 : SEG])
                    seg_carries.append((segc_s, segc_q))

                    # ---- segment finalize in t-major [128, F]
                    d_sq = dram.tile([2 * SEG], F32, tag="d_sq")
                    d_sq2 = d_sq.rearrange("(s t) -> s t", s=2)
                    nc.sync.dma_start(out=d_sq2[0:1, :], in_=srow)
                    nc.sync.dma_start(out=d_sq2[1:2, :], in_=qrow)
                    tm2 = tmaj.tile([128, 2, F], F32, tag="tm2")
                    nc.sync.dma_start(
                        out=tm2, in_=d_sq.rearrange("(s p f) -> p s f", s=2, p=128)
                    )
                    s_tm = tm2[:, 0, :]
                    q_tm = tm2[:, 1, :]
                    invp_s = invp_t[:, s * F : (s + 1) * F]
                    invm_s = invm_t[:, s * F : (s + 1) * F]
                    nmean = tmaj.tile([128, F], F32, tag="nmean")
                    nc.vector.tensor_mul(out=nmean, in0=s_tm, in1=invm_s)  # -mean
                    e2 = tmaj.tile([128, F], F32, tag="e2")
                    nc.vector.tensor_mul(out=e2, in0=q_tm, in1=invp_s)
                    msq = tmaj.tile([128, F], F32, tag="msq")
                    nc.vector.tensor_mul(out=msq, in0=nmean, in1=nmean)
                    var = tmaj.tile([128, F], F32, tag="var")
                    nc.vector.tensor_sub(out=var, in0=e2, in1=msq)
                    nc.vector.tensor_scalar_max(out=var, in0=var, scalar1=0.0)
                    sd = tmaj.tile([128, F], F32, tag="sd")
                    nc.scalar.activation(
                        out=sd, in_=var, func=mybir.ActivationFunctionType.Sqrt,
                        bias=eps_t, scale=1.0,
                    )
                    tmo = tmaj.tile([128, 2, F], F32R, tag="tmo")
                    with nc.allow_low_precision(
                        reason="f32r rounding feeds PE broadcast matmuls"
                    ):
                        nc.vector.reciprocal(out=tmo[:, 0, :], in_=sd)
                        nc.vector.tensor_mul(
                            out=tmo[:, 1, :], in0=nmean, in1=tmo[:, 0, :]
                        )
                    d_ab = dram.tile([2 * SEG], F32R, tag="d_ab")
                    nc.sync.dma_start(
                        out=d_ab.rearrange("(s p f) -> p s f", s=2, p=128), in_=tmo
                    )
                    invrow = abrow.tile([1, SEG], F32R, tag="invrow")
                    nminvrow = abrow.tile([1, SEG], F32R, tag="nminvrow")
                    d_ab2 = d_ab.rearrange("(s t) -> s t", s=2)
                    nc.sync.dma_start(out=invrow, in_=d_ab2[0:1, :])
                    nc.sync.dma_start(out=nminvrow, in_=d_ab2[1:2, :])

                    # ---- normalize segment in place: y = x*A + B
                    for j in range(NTN):
                        ps_a = pab.tile([128, TN], F32, tag="ps_a")
                        ps_b = pab.tile([128, TN], F32, tag="ps_b")
                        nc.tensor.matmul(
                            out=ps_a, lhsT=ones_r,
                            rhs=invrow[0:1, j * TN : (j + 1) * TN],
                            start=True, stop=True,
                        )
                        nc.tensor.matmul(
                            out=ps_b, lhsT=ones_r,
                            rhs=nminvrow[0:1, j * TN : (j + 1) * TN],
                            start=True, stop=True,
                        )
                        rep_a = bass.AP(
                            tensor=ps_a.tensor, offset=ps_a.offset,
                            ap=[ps_a.ap[0], [0, CB], ps_a.ap[1]],
                        )
                        rep_b = bass.AP(
                            tensor=ps_b.tensor, offset=ps_b.offset,
                            ap=[ps_b.ap[0], [0, CB], ps_b.ap[1]],
                        )
                        xs = xt[:, :, j * TN : (j + 1) * TN]
                        nc.vector.tensor_mul(out=xs, in0=xs, in1=rep_a)
                        nc.vector.tensor_add(out=xs, in0=xs, in1=rep_b)
                        if wb_general:
                            for cb in range(CB):
                                nc.scalar.activation(
                                    out=xs[:, cb, :], in_=xs[:, cb, :],
                                    func=mybir.ActivationFunctionType.Copy,
                                    bias=0.0, scale=wcol[:, cb : cb + 1],
                                )
                                nc.vector.tensor_scalar_add(
                                    out=xs[:, cb, :], in0=xs[:, cb, :],
                                    scalar1=bcol[:, cb : cb + 1],
                                )
                    for cb in range(CB):
                        nc.sync.dma_start(
                            out=y_r[cb, :, s * SEG : (s + 1) * SEG], in_=xt[:, cb, :]
                        )

    nc.finalize()
    return nc


def _get_kernel(wb_general: bool):
    if wb_general not in _CACHE:
        _CACHE[wb_general] = _build(wb_general)
    return _CACHE[wb_general]


def _make_in_maps(x, weight, bias):
    wb_general = not (np.all(weight == 1.0) and np.all(bias == 0.0))
    w_row = np.ascontiguousarray(weight.reshape(1, C).astype(np.float32))
    b_row = np.ascontiguousarray(bias.reshape(1, C).astype(np.float32))
    in_maps = []
    for core in range(NCORES):
        b_idx, h = core // 2, core % 2
        xc = np.ascontiguousarray(x[b_idx, :, h * TH : (h + 1) * TH])
        xp = np.ascontiguousarray(x[b_idx, :, 0:TH]) if h == 1 else xc
        flag = np.full((2, 1), float(h), np.float32)
        # invn[p, s*F + f] = 1 / (C * (h*TH + s*SEG + p*F + f + 1))
        t_local = (
            np.arange(NSEG)[:, None, None] * SEG
            + np.arange(128)[None, :, None] * F
            + np.arange(F)[None, None, :]
        )
        t_global = h * TH + t_local  # [NSEG, 128, F]
        invn = (1.0 / (C * (t_global.astype(np.float64) + 1.0))).astype(np.float32)
        invn = np.ascontiguousarray(invn.transpose(1, 0, 2).reshape(128, NSEG * F))
        in_maps.append(
            {
                "xc": xc, "xp": xp, "flag": flag,
                "invp": invn, "invm": np.ascontiguousarray(-invn),
                "w": w_row, "b": b_row,
            }
        )
    return in_maps, wb_general


def kernel(x, weight, bias, _trace=False, _tmpdir=None):
    x = np.asarray(x, np.float32)
    weight = np.asarray(weight, np.float32)
    bias = np.asarray(bias, np.float32)
    in_maps, wb_general = _make_in_maps(x, weight, bias)
    nc = _get_kernel(wb_general)
    res = run_bass_kernel_spmd(
        nc, in_maps, list(range(NCORES)), trace=_trace, tmpdir=_tmpdir
    )
    y = np.empty((B, C, T), np.float32)
    for core in range(NCORES):
        b_idx, h = core // 2, core % 2
        y[b_idx, :, h * TH : (h + 1) * TH] = res.results[core]["y"]
    if _trace:
        return y, res
    return y
